# revision 13
# baseline (speedup 1.0000x reference)
"""Trainium2 Bass kernel for 7x7 sliding-window self-similarity attention.

out[b,c,h,w] = sum_j softmax_j(x[h,w] * x[h+dh,w+dw]) * x[h+dh,w+dw]
over the 7x7 neighborhood (zero padding, pad=3).

Sharding: B*C = 256 independent 128x128 images, 32 images per core on 8
NeuronCores (pure data parallel, no collectives).

Per-core layout: partition p = rowblock(0..3)*32 + image(0..31); each
partition holds a 44-row x 140-col zero-padded fp32 slab (32-row block +
6-row halo, 128 cols + 6-col pad) flattened to 6160 contiguous floats, so
every 7x7 window shift is a flat offset view. All elementwise ops run on
fully contiguous 1D runs covering the pad columns too (pad results are
finite garbage that lands only in pad positions, never read).

Score symmetry: e_{-d}[i] == e_d[i-d], so only 25 canonical score/exp
tiles are computed (on an extended halo region); mirrored 24 are views.
"""

import numpy as np

import concourse.bacc as bacc
import concourse.bass as bass  # noqa: F401
import concourse.tile as tile
from concourse import mybir
from concourse.bass_utils import run_bass_kernel_spmd

N_CORES = 8
F32 = mybir.dt.float32
MULT = mybir.AluOpType.mult
ADD = mybir.AluOpType.add

B, C, H, W = 4, 64, 128, 128
N_IMG_TOTAL = B * C          # 256 independent images
IMG_PER_CORE = N_IMG_TOTAL // N_CORES  # 32
RB_N = 4                     # rowblocks per image
PAD = 6                      # host-side zero pad on each spatial side

# delta -> which accumulation team (engine) handles it; GPS is ~2x slower
# than DVE so it gets ~1/3 of the pairs.
N_GPS_PAIRS = 8


def canonical_offsets():
    """(0,0) plus one representative of each +-delta pair: 25 total."""
    canon = [(0, 0)]
    canon += [(0, dj) for dj in range(1, 4)]
    canon += [(di, dj) for di in range(1, 4) for dj in range(-3, 4)]
    return canon


def build_nc(n_img=IMG_PER_CORE, h=H, w=W):
    """Build the single-core Bass program (SPMD across 8 cores)."""
    br = h // RB_N               # rows per block (32)
    wp = w + 2 * PAD             # 140 (row stride)
    slab = br + 2 * PAD          # 44 stored rows per partition
    P = n_img * RB_N             # partitions used (128)

    nx = slab * wp               # valid x floats per partition (6160)
    le = (br + 6) * wp + 8       # extended score/exp run (5328)
    soff = 3 * wp - 4            # x offset of the extended run start
    la = br * wp                 # accumulation run (4480)
    e0_off = 3 * wp + 4          # e-tile offset of output-region start
    xq_off = 6 * wp              # x offset of output-region start

    nc = bacc.Bacc("TRN2", target_bir_lowering=False, debug=False)
    x_in = nc.dram_tensor("x", [P, nx], F32, kind="ExternalInput")
    y_out = nc.dram_tensor("y", [P, la], F32, kind="ExternalOutput")

    with tile.TileContext(nc) as tc:
        with (
            tc.tile_pool(name="big", bufs=1) as big,
            tc.tile_pool(name="se", bufs=2) as sepool,
            tc.tile_pool(name="mm", bufs=2) as mpool,
        ):
            x = big.tile([P, nx + 8], F32, tag="x")
            acc = big.tile([P, la], F32, tag="acc")       # DVE accumulator
            acc_g = big.tile([P, la], F32, tag="accg")    # GPS accumulator
            sum_d = big.tile([P, la], F32, tag="sumd")
            sum_g = big.tile([P, la], F32, tag="sumg")

            nc.sync.dma_start(out=x[:, :nx], in_=x_in[:])
            nc.vector.memset(x[:, nx:], 0.0)

            canon = canonical_offsets()
            # split pairs between engine teams: first entry (center) on DVE
            gps_set = set()
            pairs = canon[1:]
            step = max(1, len(pairs) // max(N_GPS_PAIRS, 1))
            picked = pairs[::step][:N_GPS_PAIRS]
            gps_set = set(picked)

            first_d = True
            first_g = True
            for (di, dj) in canon:
                df = di * wp + dj
                on_gps = (di, dj) in gps_set
                eng = nc.gpsimd if on_gps else nc.vector

                e = sepool.tile([P, le], F32, tag="se")

                # scores on the extended run: s = x * shift(x, +d);
                # exp applied in place on the same tile
                eng.tensor_tensor(
                    out=e[:],
                    in0=x[:, soff:soff + le],
                    in1=x[:, soff + df:soff + df + le],
                    op=MULT,
                )
                nc.scalar.activation(
                    out=e[:], in_=e[:], func=mybir.ActivationFunctionType.Exp
                )

                my_acc = acc_g if on_gps else acc
                my_sum = sum_g if on_gps else sum_d
                my_first = first_g if on_gps else first_d

                views = [(e0_off, xq_off + df)]
                if (di, dj) != (0, 0):
                    views.append((e0_off - df, xq_off - df))
                for (eo, xo) in views:
                    ev = e[:, eo:eo + la]
                    xv = x[:, xo:xo + la]
                    if my_first:
                        eng.tensor_tensor(out=my_acc[:], in0=ev, in1=xv, op=MULT)
                        eng.tensor_copy(my_sum[:], ev)
                        my_first = False
                        if on_gps:
                            first_g = False
                        else:
                            first_d = False
                        continue
                    m = mpool.tile([P, la], F32, tag="m")
                    eng.tensor_tensor(out=m[:], in0=ev, in1=xv, op=MULT)
                    eng.tensor_tensor(out=my_acc[:], in0=my_acc[:], in1=m[:],
                                      op=ADD)
                    eng.tensor_tensor(out=my_sum[:], in0=my_sum[:], in1=ev,
                                      op=ADD)

            # merge team accumulators
            nc.vector.tensor_tensor(out=acc[:], in0=acc[:], in1=acc_g[:], op=ADD)
            nc.gpsimd.tensor_tensor(out=sum_d[:], in0=sum_d[:], in1=sum_g[:],
                                    op=ADD)

            r = mpool.tile([P, la], F32, tag="m")
            scr = mpool.tile([P, la], F32, tag="m")
            nc.vector.reciprocal_approx_accurate(
                out=r[:], in_=sum_d[:], scratch=scr[:]
            )
            nc.vector.tensor_tensor(out=acc[:], in0=acc[:], in1=r[:], op=MULT)

            nc.sync.dma_start(out=y_out[:], in_=acc[:])
    nc.compile()
    return nc


_NC_CACHE = {}


def _get_nc():
    if "nc" not in _NC_CACHE:
        _NC_CACHE["nc"] = build_nc()
    return _NC_CACHE["nc"]


def make_slabs(imgs, h=H, w=W):
    """[n,h,w] fp32 -> [n*4, 44*140] slab layout (p = rb*n + img)."""
    n = imgs.shape[0]
    br = h // RB_N
    slab = br + 2 * PAD
    xp = np.pad(imgs, ((0, 0), (PAD, PAD), (PAD, PAD)))
    rows = (np.arange(RB_N) * br)[:, None] + np.arange(slab)  # [4, 44]
    sl = xp[:, rows, :]                 # [n, 4, 44, wp]
    sl = sl.transpose(1, 0, 2, 3)       # [4, n, 44, wp]
    return np.ascontiguousarray(sl.reshape(RB_N * n, -1))


def unslab_out(y, n_img, h=H, w=W):
    """[n*4, br*wp full-width] -> [n, h, w] (strip pad cols)."""
    br = h // RB_N
    wp = w + 2 * PAD
    y = y.reshape(RB_N, n_img, br, wp)[:, :, :, PAD:PAD + w]
    y = y.transpose(1, 0, 2, 3)
    return np.ascontiguousarray(y.reshape(n_img, h, w))


def run(x, **spmd_kwargs):
    """Run on 8 cores; returns (full output, BassKernelResults)."""
    nc = _get_nc()
    imgs = np.ascontiguousarray(np.asarray(x).reshape(N_IMG_TOTAL, H, W))
    imgs = imgs.astype(np.float32, copy=False)
    in_maps = [
        {"x": make_slabs(imgs[i * IMG_PER_CORE:(i + 1) * IMG_PER_CORE])}
        for i in range(N_CORES)
    ]
    res = run_bass_kernel_spmd(nc, in_maps, core_ids=list(range(N_CORES)),
                               **spmd_kwargs)
    out = np.concatenate(
        [unslab_out(res.results[i]["y"], IMG_PER_CORE) for i in range(N_CORES)],
        axis=0,
    )
    return out.reshape(B, C, H, W).astype(np.float32, copy=False), res


def kernel(x):
    out, _ = run(x)
    return out


# revision 18
# speedup vs baseline: 2.0337x; 2.0337x over previous
"""Trainium2 Bass kernel for 7x7 sliding-window self-similarity attention.

out[b,c,h,w] = sum_j softmax_j(x[h,w] * x[h+dh,w+dw]) * x[h+dh,w+dw]
over the 7x7 neighborhood (zero padding, pad=3).

Sharding: B*C = 256 independent 128x128 images, 32 images per core on 8
NeuronCores (pure data parallel, no collectives).

Per-core layout: partition p = rowblock(0..3)*32 + image(0..31); each
partition holds a 44-row x 140-col zero-padded fp32 slab (6160 contiguous
floats), so every 7x7 shift is a flat offset view. Elementwise ops run on
fully contiguous 1D runs spanning the pad columns (finite garbage there,
never read).

Score symmetry: e_{-d}[i] == e_d[i-d]; only 25 canonical score tiles are
computed on an extended halo run; mirrored views are flat offsets.

Numerator trick: acc[i] = sum_d e_d[i]*x[i+d] = (sum_d e_d*s_d views)/x[i]
with s_d = x*x_shift the score itself, so t_d = e_d*s_d gives both the
+d and -d numerator contributions as *views of one tile*; the final
division by x cancels exactly (out = acc_t / (x * sum_e)).

Engines: DVE does score/t products and the acc_t chain; ACT does exp;
TensorE accumulates sum_e into PSUM via fp32 identity matmuls (its own
SBUF ports - no DVE contention); GpSimd stays idle (its SBUF port is
shared with DVE's second read port - concurrency is net-negative).
"""

import numpy as np

import concourse.bacc as bacc
import concourse.bass as bass  # noqa: F401
import concourse.tile as tile
from concourse import mybir
from concourse.bass_utils import run_bass_kernel_spmd

N_CORES = 8
F32 = mybir.dt.float32
MULT = mybir.AluOpType.mult
ADD = mybir.AluOpType.add

B, C, H, W = 4, 64, 128, 128
N_IMG_TOTAL = B * C
IMG_PER_CORE = N_IMG_TOTAL // N_CORES  # 32
RB_N = 4
PAD = 6
MM_CHUNK = 512                # one PSUM bank of fp32


def canonical_offsets():
    canon = [(0, 0)]
    canon += [(0, dj) for dj in range(1, 4)]
    canon += [(di, dj) for di in range(1, 4) for dj in range(-3, 4)]
    return canon


def view2d(ap, off, rows, cols, stride):
    """Strided [rows, cols] view at element offset `off` of a flat [P, L] AP."""
    a = ap.copy()
    pair_t = type(a.ap)
    part = list(a.ap)[0]
    a.ap = pair_t([list(part), [stride, rows], [1, cols]])
    a.offset = a.offset + off
    return a


def build_nc(n_img=IMG_PER_CORE, h=H, w=W):
    br = h // RB_N               # 32
    wp = w + 2 * PAD             # 140
    slab = br + 2 * PAD          # 44
    P = n_img * RB_N             # 128

    nx = slab * wp               # 6160
    le = (br + 6) * wp + 8       # 5328 extended run
    soff = 3 * wp - 4            # x offset of extended run
    la = br * wp                 # 4480 full-width accumulation run
    lc = br * w                  # 4096 compact output
    t0_off = 3 * wp + 4          # t/e-tile offset of output-region start
    xq_off = 6 * wp              # x offset of output region (full width)
    mm_chunk = min(MM_CHUNK, lc)
    n_chunks = lc // mm_chunk    # 8
    rows_per_chunk = mm_chunk // w  # 4

    nc = bacc.Bacc("TRN2", target_bir_lowering=False, debug=False)
    x_in = nc.dram_tensor("x", [P, nx], F32, kind="ExternalInput")
    id_in = nc.dram_tensor("ident", [P, P], F32, kind="ExternalInput")
    y_out = nc.dram_tensor("y", [P, lc], F32, kind="ExternalOutput")

    canon = canonical_offsets()
    n_views = 2 * len(canon) - 1  # 49

    with tile.TileContext(nc) as tc:
        with (
            tc.tile_pool(name="big", bufs=1) as big,
            tc.tile_pool(name="sp", bufs=2) as spool,
            tc.tile_pool(name="ep", bufs=2) as epool,
            tc.tile_pool(name="tp", bufs=1) as tpool,
            tc.tile_pool(name="fin", bufs=1) as fin,
            tc.tile_pool(name="ps", bufs=1, space="PSUM") as ps,
        ):
            x = big.tile([P, nx + 8], F32, tag="x")
            ident = big.tile([P, P], F32, tag="id")
            acc = big.tile([P, la], F32, tag="acc")
            psum = ps.tile([P, lc], F32, tag="sum")

            nc.sync.dma_start(out=x[:, :nx], in_=x_in[:])
            nc.vector.memset(x[:, nx:], 0.0)
            nc.sync.dma_start(out=ident[:], in_=id_in[:])

            vidx = 0  # global view counter for matmul start/stop
            for k, (di, dj) in enumerate(canon):
                df = di * wp + dj
                s = spool.tile([P, le], F32, tag="s")
                e = epool.tile([P, le], F32, tag="e")

                nc.vector.tensor_tensor(
                    out=s[:],
                    in0=x[:, soff:soff + le],
                    in1=x[:, soff + df:soff + df + le],
                    op=MULT,
                )
                nc.scalar.activation(
                    out=e[:], in_=s[:], func=mybir.ActivationFunctionType.Exp
                )

                # numerator: t = e * s; both +d and -d contributions are
                # contiguous views of t
                t = tpool.tile([P, le], F32, tag="t")
                nc.vector.tensor_tensor(out=t[:], in0=e[:], in1=s[:], op=MULT)

                offs = [t0_off]
                if (di, dj) != (0, 0):
                    offs.append(t0_off - df)
                for oi, to in enumerate(offs):
                    tv = t[:, to:to + la]
                    if k == 0:
                        nc.vector.tensor_copy(acc[:], tv)
                    else:
                        nc.vector.tensor_tensor(out=acc[:], in0=acc[:],
                                                in1=tv, op=ADD)

                # denominator on PE: psum[c] += I @ e_view_chunk
                for to in offs:
                    eo = to + PAD  # image-col region of this view
                    for ci in range(n_chunks):
                        mv = view2d(e[:], eo + ci * rows_per_chunk * wp,
                                    rows_per_chunk, w, wp)
                        nc.tensor.matmul(
                            psum[:, ci * mm_chunk:(ci + 1) * mm_chunk],
                            ident[:], mv,
                            start=(vidx == 0), stop=(vidx == n_views - 1),
                        )
                    vidx += 1

            # out = acc_t / (x * sum_e)  (the x factors cancel exactly)
            den = fin.tile([P, lc], F32, tag="den")
            r = fin.tile([P, lc], F32, tag="r")
            scr = fin.tile([P, lc], F32, tag="scr")
            xc = view2d(x[:], xq_off + PAD, br, w, wp)
            nc.vector.tensor_tensor(out=den[:], in0=psum[:], in1=xc, op=MULT)
            nc.vector.reciprocal_approx_accurate(
                out=r[:], in_=den[:], scratch=scr[:]
            )
            out_c = fin.tile([P, lc], F32, tag="den")
            av = view2d(acc[:], PAD, br, w, wp)
            nc.vector.tensor_tensor(out=out_c[:], in0=av, in1=r[:], op=MULT)

            nc.sync.dma_start(out=y_out[:], in_=out_c[:])
    nc.compile()
    return nc


_NC_CACHE = {}


def _get_nc():
    if "nc" not in _NC_CACHE:
        _NC_CACHE["nc"] = build_nc()
    return _NC_CACHE["nc"]


def make_slabs(imgs, h=H, w=W):
    """[n,h,w] fp32 -> [n*4, 44*140] slab layout (p = rb*n + img)."""
    n = imgs.shape[0]
    br = h // RB_N
    slab = br + 2 * PAD
    xp = np.pad(imgs, ((0, 0), (PAD, PAD), (PAD, PAD)))
    rows = (np.arange(RB_N) * br)[:, None] + np.arange(slab)
    sl = xp[:, rows, :]
    sl = sl.transpose(1, 0, 2, 3)
    return np.ascontiguousarray(sl.reshape(RB_N * n, -1))


def unslab_out(y, n_img, h=H, w=W):
    """[n*4, br*w compact] -> [n, h, w]."""
    br = h // RB_N
    y = y.reshape(RB_N, n_img, br, w).transpose(1, 0, 2, 3)
    return np.ascontiguousarray(y.reshape(n_img, h, w))


def run(x, **spmd_kwargs):
    nc = _get_nc()
    imgs = np.ascontiguousarray(np.asarray(x).reshape(N_IMG_TOTAL, H, W))
    imgs = imgs.astype(np.float32, copy=False)
    ident = np.eye(128, dtype=np.float32)
    in_maps = [
        {"x": make_slabs(imgs[i * IMG_PER_CORE:(i + 1) * IMG_PER_CORE]),
         "ident": ident}
        for i in range(N_CORES)
    ]
    res = run_bass_kernel_spmd(nc, in_maps, core_ids=list(range(N_CORES)),
                               **spmd_kwargs)
    out = np.concatenate(
        [unslab_out(res.results[i]["y"], IMG_PER_CORE) for i in range(N_CORES)],
        axis=0,
    )
    return out.reshape(B, C, H, W).astype(np.float32, copy=False), res


def kernel(x):
    out, _ = run(x)
    return out
